# revision 4
# baseline (speedup 1.0000x reference)
"""CMC@5 retrieval-accuracy kernel v3 for Trainium2 (8 NeuronCores).

Count formulation (no argsort, no full-row top-k, no bias matmul):

  v_ij = q_i.e_j - ||e_j||^2/2          (monotone decreasing in distance)
  m_i  = max same-class v (j != i)      (from prepacked class-window matmuls)
  flag_i <=> #{j : v_ij > m_i} <= 5     (self counts as 1; the same-class
                                         argmax is excluded by a DELTA margin)

Per (query-tile, chunk-pair) PSUM holds raw dots (two K=128 fp16 passes).
One fused scalar_tensor_tensor per 1024-wide pair does everything:

  out   = (dots - m[p]) > (b[f] + DELTA)     # m: per-partition scalar AP,
  accum = sum(out)                           # b+DELTA: broadcast f32 tile

m_i comes from a per-qt window matmul over host-prepacked same-class
candidate columns (8 groups x u <= 512), combined with a host mask+bias
tile (WMB = additive_mask - b, f32, bit-consistent with the count-side b)
and max8'd: wt8[:,1] is the best same-class score, used directly as the
STT scalar. Count ops run DVE-direct from PSUM; a fraction run on GpSimd
from ScalarE fp16 copies to balance engines.

Host: sorts candidates by label, prepacks windows, runs 8 cores SPMD,
computes flags = (cnt <= 5.5) and the mean.
"""

import numpy as np
import ml_dtypes

import concourse.bass as bass
import concourse.mybir as mybir
from concourse import bacc
from concourse.tile import TileContext
from concourse.bass_utils import run_bass_kernel_spmd

N = 16384
D = 256
NCORES = 8
P = 128
CH = 512            # candidate chunk (one PSUM bank of f32)
PAIR = 2 * CH       # count granularity (two PSUM banks)
DELTA = 3e-4        # count margin: excludes the same-class argmax residual
GPS_MOD = 0         # GpSimd STT is rejected by walrus codegen (Pool engine);
                    # 0 = all count ops run DVE-direct from PSUM
SIGN_QUADS = (1, 3, 5)  # q4 % 8 values routed via the ScalarE sign path


def build_nc(n, qpc, u, gps_mod=GPS_MOD, sign_quads=SIGN_QUADS):
    nch = n // CH
    npair = nch // 2
    nqt = qpc // P
    wq = 8 * u          # window width per query tile
    assert wq <= 512
    assert nch % 4 == 0

    f32 = mybir.dt.float32
    fp16 = mybir.dt.float16
    bf16 = mybir.dt.bfloat16
    GT = mybir.AluOpType.is_gt
    ADD = mybir.AluOpType.add
    SUB = mybir.AluOpType.subtract
    SIGN = mybir.ActivationFunctionType.Sign

    nc = bacc.Bacc("TRN2", target_bir_lowering=False)
    ETA = nc.dram_tensor("ETA", [P, n], fp16, kind="ExternalInput").ap()
    ETB = nc.dram_tensor("ETB", [P, n], fp16, kind="ExternalInput").ap()
    BD1 = nc.dram_tensor("BD1", [1, n], f32, kind="ExternalInput").ap()
    QTA = nc.dram_tensor("QTA", [P, qpc], fp16, kind="ExternalInput").ap()
    QTB = nc.dram_tensor("QTB", [P, qpc], fp16, kind="ExternalInput").ap()
    EWA = nc.dram_tensor("EWA", [P, nqt * wq], fp16, kind="ExternalInput").ap()
    EWB = nc.dram_tensor("EWB", [P, nqt * wq], fp16, kind="ExternalInput").ap()
    WM8 = nc.dram_tensor("WM8", [P, nqt * wq], bf16, kind="ExternalInput").ap()
    WW3 = nc.dram_tensor("WW3", [3, nqt * wq], bf16, kind="ExternalInput").ap()
    B3R3 = nc.dram_tensor("B3R3", [3, n], bf16, kind="ExternalInput").ap()
    NEG1 = nc.dram_tensor("NEG1", [3, P], bf16, kind="ExternalInput").ap()
    OUT = nc.dram_tensor("OUT", [P, nqt * 2], f32, kind="ExternalOutput").ap()

    with TileContext(nc) as tc:
        with tc.tile_pool(name="const", bufs=1) as constp, \
             tc.tile_pool(name="qtp", bufs=2) as qtp, \
             tc.tile_pool(name="smallp", bufs=2) as smallp, \
             tc.tile_pool(name="v16p", bufs=3) as v16p, \
             tc.tile_pool(name="pairp", bufs=4, space="PSUM") as pairp:

            for s in (8, 4, 2, 1):
                if n % (s * PAIR) == 0:
                    nsplit = s
                    break
            nsub = n // nsplit
            eta_t = [constp.tile([P, nsub], fp16, tag=f"eta{i}", name=f"eta{i}")
                     for i in range(nsplit)]
            etb_t = [constp.tile([P, nsub], fp16, tag=f"etb{i}", name=f"etb{i}")
                     for i in range(nsplit)]
            bd_t = [constp.tile([P, nsub], f32, tag=f"bd{i}", name=f"bd{i}")
                    for i in range(nsplit)]
            mbr_t = [constp.tile([P, nsub], bf16, tag=f"mbr{i}", name=f"mbr{i}")
                     for i in range(nsplit)] if sign_quads else []
            neg1s = constp.tile([P, P], bf16, tag="neg1s", name="neg1s")
            junk16 = constp.tile([P, PAIR], fp16, tag="junk16")
            junk16s = constp.tile([P, PAIR], fp16, tag="junk16s")
            outsb = constp.tile([P, nqt * 2], f32, tag="outsb")
            cpt = nsub // CH

            def eta(c):
                return eta_t[c // cpt][:, (c % cpt)*CH:(c % cpt + 1)*CH]

            def etb(c):
                return etb_t[c // cpt][:, (c % cpt)*CH:(c % cpt + 1)*CH]

            def bd(p):
                c = 2 * p
                t = bd_t[c // cpt]
                off = (c % cpt) * CH
                return t[:, off:off + PAIR]

            def mbr(c, g):
                t = mbr_t[c // cpt]
                off = (c % cpt) * CH
                return t[32*g:32*g+3, off:off + CH]

            # window pass for qt: m = 2nd largest same-class score.
            # Emitted one qt AHEAD of the count loop so the DVE never waits
            # on a window chain at a qt boundary.
            qa_t = [None] * nqt
            qb_t = [None] * nqt
            wt8_t = [None] * nqt
            nmd_t = [None] * nqt

            def emit_window(qt):
                qa = qtp.tile([P, P], fp16, tag="qa")
                qb = qtp.tile([P, P], fp16, tag="qb")
                ewa = qtp.tile([P, wq], fp16, tag="ewa")
                ewb = qtp.tile([P, wq], fp16, tag="ewb")
                wm8 = qtp.tile([P, wq], bf16, tag="wm8")
                ww3 = qtp.tile([3, wq], bf16, tag="ww3")
                ws = slice(qt*wq, (qt+1)*wq)
                nc.sync.dma_start(out=qa, in_=QTA[:, qt*P:(qt+1)*P])
                nc.sync.dma_start(out=qb, in_=QTB[:, qt*P:(qt+1)*P])
                nc.sync.dma_start(out=ewa, in_=EWA[:, ws])
                nc.sync.dma_start(out=ewb, in_=EWB[:, ws])
                nc.sync.dma_start(out=wm8, in_=WM8[:, ws])
                nc.sync.dma_start(out=ww3, in_=WW3[:, ws])
                # window PSUM borrows a pair-pool buffer (first bank only)
                pswt = pairp.tile([P, PAIR], f32, tag="pst", name="psw")
                psw = pswt[:, 0:wq]
                nc.tensor.matmul(psw, neg1s[0:3, :], ww3,
                                 start=True, stop=False)
                nc.tensor.matmul(psw, qa, ewa, start=False, stop=False)
                nc.tensor.matmul(psw, qb, ewb, start=False, stop=True)
                msk = smallp.tile([P, wq], f32, tag="msk")
                nc.vector.tensor_tensor(out=msk, in0=psw, in1=wm8, op=ADD)
                wt8 = smallp.tile([P, 8], f32, tag="wt8")
                nc.vector.max(out=wt8, in_=msk)
                if sign_quads:
                    # nmd = -(m + DELTA), the per-partition sign-path bias
                    nmd = smallp.tile([P, 2], f32, tag="nmd")
                    nc.gpsimd.tensor_scalar_add(nmd[:, 0:1], wt8[:, 1:2],
                                                DELTA)
                    nc.gpsimd.tensor_scalar_mul(nmd[:, 1:2], nmd[:, 0:1],
                                                -1.0)
                    nmd_t[qt] = nmd
                qa_t[qt], qb_t[qt], wt8_t[qt] = qa, qb, wt8

            for g in range(4 if sign_quads else 1):
                nc.sync.dma_start(out=neg1s[32*g:32*g+3, :], in_=NEG1)
            emit_window(0)
            # candidate/bias loads issued after qt0's window DMAs so the
            # pipeline starts in ~5us instead of waiting on the bulk load

            def bd1_bcast(i):
                src = BD1[0:1, i*nsub:(i+1)*nsub]
                return bass.AP(src.tensor, src.offset,
                               [[0, P]] + list(src.ap[1:]))

            # big loads spread across SWDGE queues so they stream in parallel
            for i in range(nsplit):
                nc.sync.dma_start(out=eta_t[i], in_=ETA[:, i*nsub:(i+1)*nsub])
                nc.scalar.dma_start(out=etb_t[i],
                                    in_=ETB[:, i*nsub:(i+1)*nsub])
                nc.gpsimd.dma_start(out=bd_t[i], in_=bd1_bcast(i))
                if sign_quads:
                    for g in range(4):
                        nc.sync.dma_start(out=mbr_t[i][32*g:32*g+3, :],
                                          in_=B3R3[:, i*nsub:(i+1)*nsub])
            nquad = nch // 4
            for qt in range(nqt):
                qa, qb = qa_t[qt], qb_t[qt]
                m_col = wt8_t[qt][:, 1:2]

                ct_stt = smallp.tile([P, npair], f32, tag="ct_stt")
                ct_sgn = smallp.tile([P, npair], f32, tag="ct_sgn")
                nc.vector.memset(ct_stt, 0.0)
                nc.vector.memset(ct_sgn, 0.0)

                for q4 in range(nquad):
                    is_sign = bool(sign_quads) and (q4 % 8) in sign_quads
                    pst = [pairp.tile([P, PAIR], f32, tag="pst",
                                      name=f"pst{k}") for k in range(2)]
                    cs = [4*q4 + k for k in range(4)]
                    if is_sign:
                        # 4-packed K=3 bias matmuls: psum starts at -b
                        for k, c in enumerate(cs):
                            g = c % 4
                            nc.tensor.matmul(
                                pst[k//2][:, (k % 2)*CH:(k % 2 + 1)*CH],
                                neg1s[32*g:32*g+3, :], mbr(c, g),
                                start=True, stop=False,
                                tile_position=(32*g, 0))
                    for k, c in enumerate(cs):
                        nc.tensor.matmul(pst[k//2][:, (k % 2)*CH:(k % 2+1)*CH],
                                         qa, eta(c), start=not is_sign,
                                         stop=False)
                    for k, c in enumerate(cs):
                        nc.tensor.matmul(pst[k//2][:, (k % 2)*CH:(k % 2+1)*CH],
                                         qb, etb(c), start=False, stop=True)
                    if q4 == 0 and qt + 1 < nqt:
                        emit_window(qt + 1)
                    for k in range(2):
                        pr = 2*q4 + k
                        if is_sign:
                            # ScalarE: accum += sum(sign(ps - m - DELTA))
                            nc.scalar.activation(
                                out=junk16s, in_=pst[k], func=SIGN,
                                bias=nmd_t[qt][:, 1:2], scale=1.0,
                                accum_out=ct_sgn[:, pr:pr+1])
                        else:
                            nc.vector.scalar_tensor_tensor(
                                out=junk16, in0=pst[k], scalar=m_col,
                                in1=bd(pr), op0=SUB, op1=GT,
                                accum_out=ct_stt[:, pr:pr+1])

                nc.vector.tensor_reduce(out=outsb[:, 2*qt:2*qt+1], in_=ct_stt,
                                        axis=mybir.AxisListType.X, op=ADD)
                nc.vector.tensor_reduce(out=outsb[:, 2*qt+1:2*qt+2],
                                        in_=ct_sgn,
                                        axis=mybir.AxisListType.X, op=ADD)

            nc.sync.dma_start(out=OUT, in_=outsb)
    nc.compile()
    return nc


def _bf16_split3(x64):
    b0 = x64.astype(ml_dtypes.bfloat16)
    r = x64 - b0.astype(np.float64)
    b1 = r.astype(ml_dtypes.bfloat16)
    r2 = r - b1.astype(np.float64)
    b2 = r2.astype(ml_dtypes.bfloat16)
    return b0, b1, b2


def host_prep(emb, lab, n, ncores, u):
    """Numpy preprocessing. Returns (in_maps, meta) for run_bass_kernel_spmd."""
    qpc = n // ncores
    nqt = qpc // P
    wq = 8 * u
    num_classes = int(lab.max()) + 1

    perm = np.argsort(lab, kind="stable")
    e_s = emb[perm]
    counts = np.bincount(lab, minlength=num_classes)
    starts = np.zeros(num_classes + 1, np.int64)
    starts[1:] = np.cumsum(counts)

    et16 = np.ascontiguousarray(e_s.T).astype(np.float16)    # [D, n]
    eta = et16[0:P]
    etb = et16[P:D]
    norms = (e_s.astype(np.float64) ** 2).sum(axis=1)
    b32 = (norms / 2.0).astype(np.float32)                   # [n]
    bd1 = (b32 + np.float32(DELTA)).reshape(1, n)
    b0, b1, b2 = _bf16_split3(norms / 2.0)
    b3r3 = np.stack([b0, b1, b2])                            # [3, n] bf16
    neg1 = np.full((3, P), -1.0, ml_dtypes.bfloat16)

    in_maps = []
    meta = []
    slab = lab[perm]
    for core in range(ncores):
        qidx = perm[core*qpc:(core+1)*qpc]
        q16 = et16[:, core*qpc:(core+1)*qpc]
        qta = np.ascontiguousarray(q16[0:P])
        qtb = np.ascontiguousarray(q16[P:D])

        ewa = np.zeros((P, nqt * wq), np.float16)
        ewb = np.zeros((P, nqt * wq), np.float16)
        ww3 = np.zeros((3, nqt * wq), ml_dtypes.bfloat16)
        wm8 = np.full((P, nqt * wq), -30000.0, ml_dtypes.bfloat16)
        for gl in range(qpc // 16):
            qt, gr = divmod(gl, 8)
            glab = slab[core*qpc + gl*16: core*qpc + (gl+1)*16]
            cls = np.unique(glab)
            union = np.concatenate(
                [np.arange(starts[cc], starts[cc+1]) for cc in cls])
            assert len(union) <= u, (
                f"union window {len(union)} exceeds capacity {u}")
            c0 = qt*wq + gr*u
            ewa[:, c0:c0+len(union)] = eta[:, union]
            ewb[:, c0:c0+len(union)] = etb[:, union]
            ww3[:, c0:c0+len(union)] = b3r3[:, union]
            for j in range(16):
                r = gr*16 + j
                sel = slab[union] == glab[j]
                wm8[r, c0:c0+len(union)][sel] = 0.0
        in_maps.append({
            "ETA": eta, "ETB": etb, "BD1": bd1,
            "QTA": qta, "QTB": qtb,
            "EWA": ewa, "EWB": ewb, "WM8": wm8, "WW3": ww3,
            "B3R3": b3r3, "NEG1": neg1,
        })
        meta.append(qidx)
    return in_maps, meta


def combine_counts(out, n):
    """out: [P, nqt, 2] device result -> total count per (row, qt)."""
    nquad = (n // CH) // 4
    nsp = 2 * sum(1 for q in range(nquad) if (q % 8) in SIGN_QUADS)
    return out[:, :, 0] + (nsp * PAIR + out[:, :, 1]) / 2.0


_NC_CACHE = {}


def kernel(embeddings, labels):
    emb = np.asarray(embeddings, dtype=np.float32)
    lab = np.asarray(labels).astype(np.int64)
    n = emb.shape[0]
    qpc = n // NCORES
    nqt = qpc // P

    counts = np.bincount(lab)
    u = max(64, int(-((2 * int(counts.max()) + 20) // -16)) * 16)

    in_maps, _ = host_prep(emb, lab, n, NCORES, u)

    key = (n, qpc, u)
    if key not in _NC_CACHE:
        _NC_CACHE[key] = build_nc(n, qpc, u)
    nc = _NC_CACHE[key]

    res = run_bass_kernel_spmd(nc, in_maps, core_ids=list(range(NCORES)))
    total = 0.0
    for core in range(NCORES):
        om = res.results[core]["OUT"].reshape(P, nqt, 2)
        cnt = combine_counts(om, n)
        total += float((cnt <= 5.5).sum())
    return np.array(total / n, dtype=np.float32)


if __name__ == "__main__":
    rng = np.random.default_rng(0)
    emb = rng.standard_normal((N, D), dtype=np.float32)
    lab = rng.integers(0, 2048, N).astype(np.int64)
    print(kernel(emb, lab))
